# revision 17
# baseline (speedup 1.0000x reference)
"""ConvNCF Trainium2 kernel (8 NeuronCores, data-parallel over batch).

Sharding: batch 4096 -> 8 cores x 512 samples.  Per core the device batch is
1024 rows ([512 pos | 512 neg]); rows are split into 4 partition groups
g = n // 256 of 32 channels each.  Each conv layer is a single K=128
block-diagonal matmul per (tap, column-chunk): lhsT is a [128,128] fp16
4x(32x32) block-diagonal weight, so all 4 groups' convolutions run in one PE
instruction (full-array MACs, 4x fewer instructions than per-group tiling).

The host performs only the embedding row lookup (the device runtime's
indirect-DMA gather scrambles multi-row-per-partition transfers, verified
empirically) and ships 96KB of gathered fp8-e4m3 rows per core (32KB user —
deduplicated, since groups 0,2 and 1,3 share rows — plus 64KB item; fp8
costs 1e-3 end-to-end rel err vs 3e-5 for fp16, well inside tolerance);
everything else runs on device:

1. R-permute matmuls expand the 4 gathered row-groups into the conv1 im2col
   u/v factor layout upat/vpat[32g + 8a + 2b + d, (s, p)] = u[n, 2p+a-1],
   using per-matmul shifted stride-2 windows for the tap offset.
2. A broadcast tensor_tensor builds conv1 outer-product patches
   patches[pi, (s,p,q)] = upat[pi,(s,p)] * vpat[pi,(s,q)], so one K=128
   block-diag matmul per 512 columns evaluates all 16 conv1 taps (host halves
   w1 to cancel the duplicated tap rows).
3. conv2..6 read UNPADDED fp16 activation tiles with stride-2 window APs;
   out-of-range edge taps simply skip those output columns (their zero-pad
   contribution is implicit in PSUM accumulation, started by the always-valid
   (1,1) tap).  ScalarE fuses bias+relu on PSUM->SBUF evacuation.
4. Head: one block-diag matmul + fused sigmoid, fp32 out [4, 256].

Runtime: every axon RPC costs ~70ms RTT, so the steady-state path keeps the
jitted executable and all weight-derived device inputs resident across calls
and blocks exactly once per call (parallel per-shard output fetch).  Only the
gathered embedding rows (2MB fp16) and the 32KB donated output-zero buffer
move per call; the zero buffer for the next call is uploaded during this
call's execution.  Weight-derived constants are fingerprinted each call and
re-uploaded if the caller passes different weights.
"""

from concurrent.futures import ThreadPoolExecutor

import numpy as np

B, D, NFM = 4096, 64, 32
N_CORES = 8
NB = B // N_CORES          # 512 samples per core
NDEV = 2 * NB              # 1024 device rows (pos branch then neg branch)
NG = NDEV // 4             # 256 rows per partition group
N_TILES = 32
ST = NG // N_TILES         # 8 slots per group per tile

IN_SIDE = {2: 32, 3: 16, 4: 8, 5: 4, 6: 2}   # unpadded input side per layer
OUT_SIDE = {1: 32, 2: 16, 3: 8, 4: 4, 5: 2, 6: 1}


def win1d(shift, isize, osize):
    """Valid out range [lo, hi) for in index 2*o + shift in [0, isize)."""
    lo = 0
    while 2 * lo + shift < 0:
        lo += 1
    hi = osize
    while hi > lo and 2 * (hi - 1) + shift >= isize:
        hi -= 1
    return lo, hi


# conv1 u/v factor windows over the 64-wide embedding rows
WIN = [(lambda lo_hi: (lo_hi[0], lo_hi[1], 2 * lo_hi[0] + a - 1))(win1d(a - 1, 64, 32))
       for a in range(4)]


def _build_program():
    MAXL = 9  # all 6 conv layers + head (debug knob, always full network)
    import concourse.bacc as bacc
    import concourse.tile as tile
    from concourse import mybir

    F8 = mybir.dt.float8e4
    F16 = mybir.dt.float16
    F32 = mybir.dt.float32
    AF = mybir.ActivationFunctionType

    nc = bacc.Bacc("TRN2", target_bir_lowering=False, name="convncf")

    # embeddings ship as fp8 e4m3 (1e-3 end-to-end rel err, half the bytes);
    # ug is deduplicated: per-core groups 0,2 and 1,3 share user rows, so only
    # 2 row-groups upload and rmat's u-blocks fan row g%2 out to both groups
    ug_t = nc.dram_tensor("ug", [2, NG * 64], F8, kind="ExternalInput")
    vg_t = nc.dram_tensor("vg", [4, NG * 64], F8, kind="ExternalInput")
    rmat_t = nc.dram_tensor("rmat", [32, 8 * 128], F8, kind="ExternalInput")
    w1bd_t = nc.dram_tensor("w1bd", [128, 128], F16, kind="ExternalInput")
    wbd_t = nc.dram_tensor("wbd", [128, 5 * 16 * 128], F16, kind="ExternalInput")
    wpbd_t = nc.dram_tensor("wpbd", [128, 4], F16, kind="ExternalInput")
    bias_t = nc.dram_tensor("biases", [128, 8], F32, kind="ExternalInput")
    out_t = nc.dram_tensor("out", [4, NG], F32, kind="ExternalOutput")

    with tile.TileContext(nc) as tc:
        with (
            tc.tile_pool(name="const", bufs=1) as constp,
            tc.tile_pool(name="glob", bufs=1) as globp,
            tc.tile_pool(name="work", bufs=2) as workp,
            tc.tile_pool(name="ps1", bufs=2, space="PSUM") as ps1p,
            tc.tile_pool(name="ps2", bufs=2, space="PSUM") as ps2p,
            tc.tile_pool(name="ps3", bufs=2, space="PSUM") as ps3p,
        ):
            w1bd = constp.tile([128, 128], F16, name="w1bd")
            wbd = constp.tile([128, 5 * 16 * 128], F16, name="wbd")
            wpbd = constp.tile([128, 4], F16, name="wpbd")
            biases = constp.tile([128, 8], F32, name="biases")
            upat = globp.tile([128, NG * 32], F16, name="upat")
            vpat = globp.tile([128, NG * 32], F16, name="vpat")
            x5 = globp.tile([128, NG * 16], F16, name="x5")   # conv5 in, 4x4
            x6 = globp.tile([128, NG * 4], F16, name="x6")    # conv6 in, 2x2
            y6 = globp.tile([128, NG], F16, name="y6")
            outsb = globp.tile([4, NG], F32, name="outsb")

            nc.gpsimd.memset(y6[:], 0.0)
            nc.sync.dma_start(w1bd[:], w1bd_t[:])
            nc.sync.dma_start(wbd[:], wbd_t[:])
            nc.sync.dma_start(wpbd[:], wpbd_t[:])
            nc.sync.dma_start(biases[:], bias_t[:])

            # ---- R-permute into upat/vpat (staging freed afterwards) ----
            with tc.tile_pool(name="pre", bufs=1) as prep:
                rmat = prep.tile([32, 8 * 128], F8, name="rmat")
                stg = prep.tile([128, NG * 64], F8, name="stg")
                nc.sync.dma_start(rmat[:], rmat_t[:])
                nc.gpsimd.memset(stg[:], 0.0)
                st3 = stg[:].rearrange("c (s e) -> c s e", e=64)
                SCH = 16  # slots per psum chunk -> 512 cols
                order = [1, 0, 2, 3]
                for tbl in range(2):
                    if tbl == 0:
                        nc.sync.dma_start(stg[0:2, :], ug_t[:])
                    else:
                        nc.sync.dma_start(stg[0:4, :], vg_t[:])
                    dstp = upat if tbl == 0 else vpat
                    for ch in range(NG // SCH):
                        s0 = ch * SCH
                        ps = ps2p.tile([128, 512], F32, tag="ps2", name="psr")
                        for i, t in enumerate(order):
                            lo, hi, o = WIN[t]
                            rhs = st3[
                                0:32, s0 : s0 + SCH, o : o + 2 * (hi - lo) - 1 : 2
                            ]
                            dst = ps[:].rearrange("c (s q) -> c s q", q=32)[
                                :, :, lo:hi
                            ]
                            nc.tensor.matmul(
                                dst,
                                rmat[
                                    :,
                                    128 * (4 * tbl + t) : 128 * (4 * tbl + t) + 128,
                                ],
                                rhs,
                                start=(i == 0),
                                stop=(i == 3),
                            )
                        nc.scalar.activation(
                            dstp[:, s0 * 32 : (s0 + SCH) * 32], ps[:], AF.Copy
                        )

            upat3 = upat[:].rearrange("c (s q) -> c s q", q=32)
            vpat3 = vpat[:].rearrange("c (s q) -> c s q", q=32)

            def w_l(layer, t):  # layer 2..6, tap t=4a+b -> [128,128] blockdiag
                c0 = ((layer - 2) * 16 + t) * 128
                return wbd[:, c0 : c0 + 128]

            # tap emission order: always-valid tap (a=1,b=1) first (start=True)
            TAP_ORDER = [5] + [t for t in range(16) if t != 5]

            def conv_layer(layer, xin, xout, psp, pstag, glob_s0=None, st=ST):
                """One block-diag K=128 matmul per (tap, chunk); windowed
                edge taps skip out-of-range columns."""
                isz = IN_SIDE[layer]
                osz = OUT_SIDE[layer]
                cols_slot = osz * osz
                total = st * cols_slot
                chw = min(total, 512)
                slots_ch = max(1, chw // cols_slot)
                nch = (total + chw - 1) // chw
                xi = xin[:].rearrange("c (s i) -> c s i", i=isz * isz)
                for ch in range(nch):
                    sa = ch * slots_ch
                    ps = psp.tile([128, chw], F32, tag=pstag, name="psc")
                    ps3 = ps[:].rearrange("c (s p q) -> c s p q", s=slots_ch, p=osz)
                    taps = []
                    for t in TAP_ORDER:
                        a, b = t // 4, t % 4
                        plo, phi = win1d(a - 1, isz, osz)
                        qlo, qhi = win1d(b - 1, isz, osz)
                        if plo < phi and qlo < qhi:
                            taps.append((t, a, b, plo, phi, qlo, qhi))
                    for i, (t, a, b, plo, phi, qlo, qhi) in enumerate(taps):
                        po = 2 * plo + a - 1
                        qo = 2 * qlo + b - 1
                        rhs = xi[:, sa : sa + slots_ch, :].rearrange(
                            "c s (p q) -> c s p q", p=isz
                        )[
                            :,
                            :,
                            po : po + 2 * (phi - plo) - 1 : 2,
                            qo : qo + 2 * (qhi - qlo) - 1 : 2,
                        ]
                        nc.tensor.matmul(
                            ps3[:, :, plo:phi, qlo:qhi],
                            w_l(layer, t),
                            rhs,
                            start=(i == 0),
                            stop=(i == len(taps) - 1),
                        )
                    base = (glob_s0 + sa) if glob_s0 is not None else sa
                    dst = xout[
                        :, base * (osz * osz) : (base + slots_ch) * (osz * osz)
                    ]
                    nc.scalar.activation(
                        dst,
                        ps[:],
                        AF.Relu,
                        bias=biases[:, layer - 1 : layer],
                    )

            # ---------------- tiled conv1..conv4 ----------------
            for ti in range(N_TILES):
                s0 = ti * ST
                patches = workp.tile(
                    [128, ST * 1024], F16, tag="patches", name="patches", bufs=1
                )
                x2 = workp.tile([128, ST * 1024], F16, tag="x2", name="x2")
                x3 = workp.tile([128, ST * 256], F16, tag="x3", name="x3", bufs=1)
                x4 = workp.tile([128, ST * 64], F16, tag="x4", name="x4", bufs=1)

                pat4 = patches[:].rearrange("c (s p q) -> c s p q", p=32, q=32)
                u_in = upat3[:, s0 : s0 + ST, :].unsqueeze(3).broadcast_to(
                    [128, ST, 32, 32]
                )
                v_in = vpat3[:, s0 : s0 + ST, :].unsqueeze(2).broadcast_to(
                    [128, ST, 32, 32]
                )
                nc.vector.tensor_tensor(pat4, u_in, v_in, mybir.AluOpType.mult)

                # conv1: K=128 block-diag matmul per 512 cols (all 16 taps)
                for half in range(ST * 2):
                    ps = ps1p.tile([128, 512], F32, tag="ps1", name="ps1t")
                    nc.tensor.matmul(
                        ps[:],
                        w1bd[:],
                        patches[:, 512 * half : 512 * (half + 1)],
                        start=True,
                        stop=True,
                    )
                    nc.scalar.activation(
                        x2[:, 512 * half : 512 * (half + 1)],
                        ps[:],
                        AF.Relu,
                        bias=biases[:, 0:1],
                    )

                if MAXL >= 2:
                    conv_layer(2, x2, x3, ps1p, "ps1")
                if MAXL >= 3:
                    conv_layer(3, x3, x4, ps2p, "ps2")
                if MAXL >= 4:
                    conv_layer(4, x4, x5, ps3p, "ps3", glob_s0=s0)

            # ---------------- conv5 + conv6 (global) ----------------
            if MAXL >= 5:
                conv_layer(5, x5, x6, ps2p, "ps2", st=NG)
            if MAXL >= 6:
                conv_layer(6, x6, y6, ps2p, "ps2", st=NG)

            # ---------------- head ----------------
            psh = ps3p.tile([128, 256], F32, tag="ps3", name="psh")
            nc.tensor.matmul(
                psh[0:4, 0:NG], wpbd[:], y6[:], start=True, stop=True
            )
            nc.scalar.activation(
                outsb[:],
                psh[0:4, 0:NG],
                AF.Sigmoid,
                bias=biases[0:4, 6:7],
            )
            nc.sync.dma_start(out_t[:], outsb[:])

    nc.compile()
    return nc


def _prep_weights(inputs):
    """Weight-derived device constants (identical on every core)."""
    import ml_dtypes

    w1 = np.asarray(inputs["conv1_w"], dtype=np.float32)
    b1 = np.asarray(inputs["conv1_b"], dtype=np.float32)
    wr = np.asarray(inputs["rest_w"], dtype=np.float32)
    br = np.asarray(inputs["rest_b"], dtype=np.float32)
    wp = np.asarray(inputs["pred_w"], dtype=np.float32)
    bp = np.asarray(inputs["pred_b"], dtype=np.float32)

    # R[src, (4*tbl + t)*128 + dst] with dst = 32g + 8a + 2b + d; the u-table
    # blocks (tbl=0) read deduplicated src row g%2, the v blocks src row g
    rmat = np.zeros((32, 8 * 128), dtype=ml_dtypes.float8_e4m3)
    for g in range(4):
        for a in range(4):
            for b in range(4):
                for dd in range(2):
                    dst = 32 * g + 8 * a + 2 * b + dd
                    rmat[g % 2, 128 * a + dst] = 1.0
                    rmat[g, 128 * (4 + b) + dst] = 1.0
    # conv1 block-diag: w1bd[32g + r, 32g' + co] = delta_gg' * w1[co,0,a,b]/2
    w1blk = np.zeros((32, 32), dtype=np.float16)  # [r=(8a+2b+d), cout]
    for a in range(4):
        for b in range(4):
            for dd in range(2):
                w1blk[8 * a + 2 * b + dd, :] = 0.5 * w1[:, 0, a, b]
    w1bd = np.zeros((128, 128), dtype=np.float16)
    for g in range(4):
        w1bd[32 * g : 32 * g + 32, 32 * g : 32 * g + 32] = w1blk
    # conv2..6 block-diag per tap
    wbd = np.zeros((128, 5 * 16 * 128), dtype=np.float16)
    for L in range(5):
        for a in range(4):
            for b in range(4):
                col0 = (L * 16 + 4 * a + b) * 128
                blkT = wr[L, :, :, a, b].T.astype(np.float16)  # [cin, cout]
                for g in range(4):
                    wbd[
                        32 * g : 32 * g + 32, col0 + 32 * g : col0 + 32 * g + 32
                    ] = blkT
    # head block-diag: wpbd[32g + c, g] = wp[0, c]
    wpbd = np.zeros((128, 4), dtype=np.float16)
    biases = np.zeros((128, 8), dtype=np.float32)
    for g in range(4):
        wpbd[32 * g : 32 * g + 32, g] = wp[0, :]
        biases[32 * g : 32 * g + 32, 0] = b1
        for L in range(5):
            biases[32 * g : 32 * g + 32, 1 + L] = br[L]
    biases[:, 6] = bp[0]
    return dict(rmat=rmat, w1bd=w1bd, wbd=wbd, wpbd=wpbd, biases=biases)


def _gather_u(inputs):
    """Global [16, NG*64] fp8 ug: rows 2c+h for core c, half h.

    Per core the device expands row h to partition groups h and h+2, so
    only the 512 distinct user rows upload.  Gathering from the fp32 table
    first and narrowing only the gathered rows avoids converting the 256MB
    user table every call.
    """
    import ml_dtypes

    idx_u = np.asarray(inputs["user"]).reshape(-1)
    uw = np.asarray(inputs["user_emb_w"])
    return (
        uw[idx_u]
        .astype(ml_dtypes.float8_e4m3)
        .reshape(2 * N_CORES, NG * 64)
    )


def _gather_v(inputs):
    """Global [32, NG*64] fp8 vg: item_pos rows for groups 0,1 of each
    core, item_neg rows for groups 2,3."""
    import ml_dtypes

    ipos = np.asarray(inputs["item_pos"]).reshape(N_CORES, 2, NG)
    ineg = np.asarray(inputs["item_neg"]).reshape(N_CORES, 2, NG)
    iw = np.asarray(inputs["item_emb_w"])
    idx_v = np.concatenate([ipos, ineg], axis=1).reshape(-1)
    return (
        iw[idx_v]
        .astype(ml_dtypes.float8_e4m3)
        .reshape(4 * N_CORES, NG * 64)
    )


_CACHED = {}
_WEIGHT_NAMES = ("rmat", "w1bd", "wbd", "wpbd", "biases")


def _get_runtime():
    """Build the Bass program + cached jitted executable once per process."""
    if "rt" in _CACHED:
        return _CACHED["rt"]

    import jax
    from jax.sharding import Mesh, NamedSharding, PartitionSpec

    import warnings

    with warnings.catch_warnings():
        warnings.simplefilter("ignore")
        try:
            from jax.experimental.shard_map import shard_map

            sm_kwargs = {"check_rep": False}
        except ImportError:
            from jax import shard_map

            sm_kwargs = {"check_vma": False}
    from concourse import mybir
    from concourse.bass2jax import (
        _bass_exec_p,
        install_neuronx_cc_hook,
        partition_id_tensor,
    )

    install_neuronx_cc_hook()
    nc = _build_program()

    partition_name = (
        nc.partition_id_tensor.name if nc.partition_id_tensor else None
    )
    in_names, out_names, out_avals, zero_shapes = [], [], [], []
    for alloc in nc.m.functions[0].allocations:
        if not isinstance(alloc, mybir.MemoryLocationSet):
            continue
        name = alloc.memorylocations[0].name
        if alloc.kind == "ExternalInput":
            if name != partition_name:
                in_names.append(name)
        elif alloc.kind == "ExternalOutput":
            out_names.append(name)
            shape = tuple(alloc.tensor_shape)
            dtype = mybir.dt.np(alloc.dtype)
            out_avals.append(jax.core.ShapedArray(shape, dtype))
            zero_shapes.append(((N_CORES * shape[0], *shape[1:]), dtype))
    n_params = len(in_names)
    n_outs = len(out_avals)
    all_names = in_names + out_names + (
        [partition_name] if partition_name else []
    )

    def _body(*args):
        operands = list(args)
        if partition_name is not None:
            operands.append(partition_id_tensor())
        outs = _bass_exec_p.bind(
            *operands,
            out_avals=tuple(out_avals),
            in_names=tuple(all_names),
            out_names=tuple(out_names),
            lowering_input_output_aliases=(),
            sim_require_finite=True,
            sim_require_nnan=True,
            nc=nc,
        )
        return tuple(outs)

    devices = jax.devices()[:N_CORES]
    assert len(devices) == N_CORES
    mesh = Mesh(np.asarray(devices), ("core",))
    sh = NamedSharding(mesh, PartitionSpec("core"))
    in_specs = (PartitionSpec("core"),) * (n_params + n_outs)
    out_specs = (PartitionSpec("core"),) * len(out_names)
    donate = tuple(range(n_params, n_params + n_outs))
    sharded = jax.jit(
        shard_map(
            _body,
            mesh=mesh,
            in_specs=in_specs,
            out_specs=out_specs,
            **sm_kwargs,
        ),
        donate_argnums=donate,
        keep_unused=True,
    )

    rt = dict(
        jax=jax,
        nc=nc,
        sharded=sharded,
        in_names=in_names,
        out_names=out_names,
        zero_shapes=zero_shapes,
        sh=sh,
        pool=ThreadPoolExecutor(max_workers=2 * N_CORES),
        weights_np=None,
        weights_dev=None,
        next_zeros=None,
    )
    _CACHED["rt"] = rt
    return rt


def _make_zeros(rt):
    jax, sh = rt["jax"], rt["sh"]
    return [
        jax.device_put(np.zeros(shape, dtype), sh)
        for shape, dtype in rt["zero_shapes"]
    ]


def _weight_hash(inputs):
    import hashlib

    h = hashlib.blake2b()
    for k in ("conv1_w", "conv1_b", "rest_w", "rest_b", "pred_w", "pred_b"):
        a = np.asarray(inputs[k])
        h.update(str(a.shape).encode())
        h.update(np.ascontiguousarray(a).tobytes())
    return h.digest()


def _run_device(inputs):
    rt = _get_runtime()
    jax, sh = rt["jax"], rt["sh"]

    # start streaming the embedding rows before anything else; u first so
    # its transfer overlaps the v gather
    ug_dev = jax.device_put(_gather_u(inputs), sh)
    vg_dev = jax.device_put(_gather_v(inputs), sh)

    # weight-derived constants stay device-resident; re-derive and re-upload
    # only when the caller passes different weight bytes
    wh = _weight_hash(inputs)
    if rt["weights_np"] != wh:
        rt["weights_np"] = None  # invalidate until the upload fully succeeds
        wnp = _prep_weights(inputs)
        rt["weights_dev"] = {
            k: jax.device_put(np.concatenate([wnp[k]] * N_CORES, axis=0), sh)
            for k in _WEIGHT_NAMES
        }
        rt["weights_np"] = wh

    per_call = {"ug": ug_dev, "vg": vg_dev}
    args = [
        per_call[name] if name in per_call else rt["weights_dev"][name]
        for name in rt["in_names"]
    ]
    zeros = rt["next_zeros"]
    rt["next_zeros"] = None  # never reuse a possibly-donated buffer on error
    if zeros is None:
        zeros = _make_zeros(rt)
    outs = rt["sharded"](*args, *zeros)
    # donated zero buffers are consumed; stage the next call's copy so its
    # upload overlaps this call's execution
    rt["next_zeros"] = _make_zeros(rt)

    # parallel per-shard fetch: one blocking point, ~RTT total
    out_global = outs[0]  # [N_CORES*4, NG] f32
    shards = out_global.addressable_shards
    datas = list(rt["pool"].map(lambda s: np.asarray(s.data), shards))
    res = np.empty((N_CORES * 4, NG), dtype=np.float32)
    for s, d in zip(shards, datas):
        r0 = s.index[0].start or 0
        res[r0 : r0 + d.shape[0]] = d
    return res


def kernel_with_stats(**inputs):
    try:
        per_core_out = _run_device(inputs).reshape(N_CORES, 4, NG)
    except Exception:
        # fall back to the stock (slow but battle-tested) runner
        from concourse.bass_utils import run_bass_kernel_spmd

        if "nc_slow" not in _CACHED:
            _CACHED["nc_slow"] = _build_program()
        wnp = _prep_weights(inputs)
        ug = _gather_u(inputs)
        vg = _gather_v(inputs)
        in_maps = [
            dict(
                ug=ug[2 * c : 2 * c + 2],
                vg=vg[4 * c : 4 * c + 4],
                **wnp,
            )
            for c in range(N_CORES)
        ]
        res = run_bass_kernel_spmd(
            _CACHED["nc_slow"], in_maps, core_ids=list(range(N_CORES))
        )
        per_core_out = np.stack([res.results[c]["out"] for c in range(N_CORES)])

    out1 = np.zeros((B, 1), dtype=np.float32)
    out2 = np.zeros((B, 1), dtype=np.float32)
    for c in range(N_CORES):
        o = per_core_out[c]  # [4, NG]
        out1[NB * c : NB * c + NB, 0] = o[0:2].reshape(-1)
        out2[NB * c : NB * c + NB, 0] = o[2:4].reshape(-1)
    return (out1, out2), None


def kernel(**inputs):
    out, _ = kernel_with_stats(**inputs)
    return out


# revision 18
# speedup vs baseline: 1.0350x; 1.0350x over previous
"""ConvNCF Trainium2 kernel (8 NeuronCores, data-parallel over batch).

Sharding: batch 4096 -> 8 cores x 512 samples.  Per core the device batch is
1024 rows ([512 pos | 512 neg]); rows are split into 4 partition groups
g = n // 256 of 32 channels each.  Each conv layer is a single K=128
block-diagonal matmul per (tap, column-chunk): lhsT is a [128,128] fp16
4x(32x32) block-diagonal weight, so all 4 groups' convolutions run in one PE
instruction (full-array MACs, 4x fewer instructions than per-group tiling).

The host performs only the embedding row lookup (the device runtime's
indirect-DMA gather scrambles multi-row-per-partition transfers, verified
empirically) and ships 96KB of gathered fp8-e4m3 rows per core (32KB user —
deduplicated, since groups 0,2 and 1,3 share rows — plus 64KB item; fp8
costs 1e-3 end-to-end rel err vs 3e-5 for fp16, well inside tolerance);
everything else runs on device:

1. R-permute matmuls expand the 4 gathered row-groups into the conv1 im2col
   u/v factor layout upat/vpat[32g + 8a + 2b + d, (s, p)] = u[n, 2p+a-1],
   using per-matmul shifted stride-2 windows for the tap offset.
2. A broadcast tensor_tensor builds conv1 outer-product patches
   patches[pi, (s,p,q)] = upat[pi,(s,p)] * vpat[pi,(s,q)], so one K=128
   block-diag matmul per 512 columns evaluates all 16 conv1 taps (host halves
   w1 to cancel the duplicated tap rows).
3. conv2..6 read UNPADDED fp16 activation tiles with stride-2 window APs;
   out-of-range edge taps simply skip those output columns (their zero-pad
   contribution is implicit in PSUM accumulation, started by the always-valid
   (1,1) tap).  ScalarE fuses bias+relu on PSUM->SBUF evacuation.
4. Head: one block-diag matmul + fused sigmoid, fp32 out [4, 256].

Runtime: every axon RPC costs ~70ms RTT, so the steady-state path keeps the
jitted executable and all weight-derived device inputs resident across calls
and blocks exactly once per call (parallel per-shard output fetch).  Only the
gathered embedding rows (2MB fp16) and the 32KB donated output-zero buffer
move per call; the zero buffer for the next call is uploaded during this
call's execution.  Weight-derived constants are fingerprinted each call and
re-uploaded if the caller passes different weights.
"""

from concurrent.futures import ThreadPoolExecutor

import numpy as np

B, D, NFM = 4096, 64, 32
N_CORES = 8
NB = B // N_CORES          # 512 samples per core
NDEV = 2 * NB              # 1024 device rows (pos branch then neg branch)
NG = NDEV // 4             # 256 rows per partition group
N_TILES = 32
ST = NG // N_TILES         # 8 slots per group per tile

IN_SIDE = {2: 32, 3: 16, 4: 8, 5: 4, 6: 2}   # unpadded input side per layer
OUT_SIDE = {1: 32, 2: 16, 3: 8, 4: 4, 5: 2, 6: 1}


def win1d(shift, isize, osize):
    """Valid out range [lo, hi) for in index 2*o + shift in [0, isize)."""
    lo = 0
    while 2 * lo + shift < 0:
        lo += 1
    hi = osize
    while hi > lo and 2 * (hi - 1) + shift >= isize:
        hi -= 1
    return lo, hi


# conv1 u/v factor windows over the 64-wide embedding rows
WIN = [(lambda lo_hi: (lo_hi[0], lo_hi[1], 2 * lo_hi[0] + a - 1))(win1d(a - 1, 64, 32))
       for a in range(4)]


def _build_program():
    MAXL = 9  # all 6 conv layers + head (debug knob, always full network)
    import concourse.bacc as bacc
    import concourse.tile as tile
    from concourse import mybir

    F8 = mybir.dt.float8e4
    F16 = mybir.dt.float16
    F32 = mybir.dt.float32
    AF = mybir.ActivationFunctionType

    nc = bacc.Bacc("TRN2", target_bir_lowering=False, name="convncf")

    # embeddings ship as fp8 e4m3 (1e-3 end-to-end rel err, half the bytes);
    # ug is deduplicated: per-core groups 0,2 and 1,3 share user rows, so only
    # 2 row-groups upload and rmat's u-blocks fan row g%2 out to both groups
    ug_t = nc.dram_tensor("ug", [2, NG * 64], F8, kind="ExternalInput")
    vg_t = nc.dram_tensor("vg", [4, NG * 64], F8, kind="ExternalInput")
    rmat_t = nc.dram_tensor("rmat", [32, 8 * 128], F8, kind="ExternalInput")
    w1bd_t = nc.dram_tensor("w1bd", [128, 128], F16, kind="ExternalInput")
    wbd_t = nc.dram_tensor("wbd", [128, 5 * 16 * 128], F16, kind="ExternalInput")
    wpbd_t = nc.dram_tensor("wpbd", [128, 4], F16, kind="ExternalInput")
    bias_t = nc.dram_tensor("biases", [128, 8], F32, kind="ExternalInput")
    out_t = nc.dram_tensor("out", [4, NG], F32, kind="ExternalOutput")

    with tile.TileContext(nc) as tc:
        with (
            tc.tile_pool(name="const", bufs=1) as constp,
            tc.tile_pool(name="glob", bufs=1) as globp,
            tc.tile_pool(name="work", bufs=2) as workp,
            tc.tile_pool(name="ps1", bufs=2, space="PSUM") as ps1p,
            tc.tile_pool(name="ps2", bufs=2, space="PSUM") as ps2p,
            tc.tile_pool(name="ps3", bufs=2, space="PSUM") as ps3p,
        ):
            w1bd = constp.tile([128, 128], F16, name="w1bd")
            wbd = constp.tile([128, 5 * 16 * 128], F16, name="wbd")
            wpbd = constp.tile([128, 4], F16, name="wpbd")
            biases = constp.tile([128, 8], F32, name="biases")
            upat = globp.tile([128, NG * 32], F16, name="upat")
            vpat = globp.tile([128, NG * 32], F16, name="vpat")
            x5 = globp.tile([128, NG * 16], F16, name="x5")   # conv5 in, 4x4
            x6 = globp.tile([128, NG * 4], F16, name="x6")    # conv6 in, 2x2
            y6 = globp.tile([128, NG], F16, name="y6")
            outsb = globp.tile([4, NG], F32, name="outsb")

            nc.gpsimd.memset(y6[:], 0.0)
            nc.sync.dma_start(w1bd[:], w1bd_t[:])
            nc.sync.dma_start(wbd[:], wbd_t[:])
            nc.sync.dma_start(wpbd[:], wpbd_t[:])
            nc.sync.dma_start(biases[:], bias_t[:])

            # ---- R-permute into upat/vpat (staging freed afterwards) ----
            with tc.tile_pool(name="pre", bufs=1) as prep:
                rmat = prep.tile([32, 8 * 128], F8, name="rmat")
                stg = prep.tile([128, NG * 64], F8, name="stg")
                nc.sync.dma_start(rmat[:], rmat_t[:])
                nc.gpsimd.memset(stg[:], 0.0)
                st3 = stg[:].rearrange("c (s e) -> c s e", e=64)
                SCH = 16  # slots per psum chunk -> 512 cols
                order = [1, 0, 2, 3]
                for tbl in range(2):
                    if tbl == 0:
                        nc.sync.dma_start(stg[0:2, :], ug_t[:])
                    else:
                        nc.sync.dma_start(stg[0:4, :], vg_t[:])
                    dstp = upat if tbl == 0 else vpat
                    for ch in range(NG // SCH):
                        s0 = ch * SCH
                        ps = ps2p.tile([128, 512], F32, tag="ps2", name="psr")
                        for i, t in enumerate(order):
                            lo, hi, o = WIN[t]
                            rhs = st3[
                                0:32, s0 : s0 + SCH, o : o + 2 * (hi - lo) - 1 : 2
                            ]
                            dst = ps[:].rearrange("c (s q) -> c s q", q=32)[
                                :, :, lo:hi
                            ]
                            nc.tensor.matmul(
                                dst,
                                rmat[
                                    :,
                                    128 * (4 * tbl + t) : 128 * (4 * tbl + t) + 128,
                                ],
                                rhs,
                                start=(i == 0),
                                stop=(i == 3),
                            )
                        nc.scalar.activation(
                            dstp[:, s0 * 32 : (s0 + SCH) * 32], ps[:], AF.Copy
                        )

            upat3 = upat[:].rearrange("c (s q) -> c s q", q=32)
            vpat3 = vpat[:].rearrange("c (s q) -> c s q", q=32)

            def w_l(layer, t):  # layer 2..6, tap t=4a+b -> [128,128] blockdiag
                c0 = ((layer - 2) * 16 + t) * 128
                return wbd[:, c0 : c0 + 128]

            # tap emission order: always-valid tap (a=1,b=1) first (start=True)
            TAP_ORDER = [5] + [t for t in range(16) if t != 5]

            def conv_layer(layer, xin, xout, psp, pstag, glob_s0=None, st=ST):
                """One block-diag K=128 matmul per (tap, chunk); windowed
                edge taps skip out-of-range columns."""
                isz = IN_SIDE[layer]
                osz = OUT_SIDE[layer]
                cols_slot = osz * osz
                total = st * cols_slot
                chw = min(total, 512)
                slots_ch = max(1, chw // cols_slot)
                nch = (total + chw - 1) // chw
                xi = xin[:].rearrange("c (s i) -> c s i", i=isz * isz)
                for ch in range(nch):
                    sa = ch * slots_ch
                    ps = psp.tile([128, chw], F32, tag=pstag, name="psc")
                    ps3 = ps[:].rearrange("c (s p q) -> c s p q", s=slots_ch, p=osz)
                    taps = []
                    for t in TAP_ORDER:
                        a, b = t // 4, t % 4
                        plo, phi = win1d(a - 1, isz, osz)
                        qlo, qhi = win1d(b - 1, isz, osz)
                        if plo < phi and qlo < qhi:
                            taps.append((t, a, b, plo, phi, qlo, qhi))
                    for i, (t, a, b, plo, phi, qlo, qhi) in enumerate(taps):
                        po = 2 * plo + a - 1
                        qo = 2 * qlo + b - 1
                        rhs = xi[:, sa : sa + slots_ch, :].rearrange(
                            "c s (p q) -> c s p q", p=isz
                        )[
                            :,
                            :,
                            po : po + 2 * (phi - plo) - 1 : 2,
                            qo : qo + 2 * (qhi - qlo) - 1 : 2,
                        ]
                        nc.tensor.matmul(
                            ps3[:, :, plo:phi, qlo:qhi],
                            w_l(layer, t),
                            rhs,
                            start=(i == 0),
                            stop=(i == len(taps) - 1),
                        )
                    base = (glob_s0 + sa) if glob_s0 is not None else sa
                    dst = xout[
                        :, base * (osz * osz) : (base + slots_ch) * (osz * osz)
                    ]
                    nc.scalar.activation(
                        dst,
                        ps[:],
                        AF.Relu,
                        bias=biases[:, layer - 1 : layer],
                    )

            # ---------------- tiled conv1..conv4 ----------------
            for ti in range(N_TILES):
                s0 = ti * ST
                patches = workp.tile(
                    [128, ST * 1024], F16, tag="patches", name="patches", bufs=1
                )
                x2 = workp.tile([128, ST * 1024], F16, tag="x2", name="x2")
                x3 = workp.tile([128, ST * 256], F16, tag="x3", name="x3", bufs=1)
                x4 = workp.tile([128, ST * 64], F16, tag="x4", name="x4", bufs=1)

                pat4 = patches[:].rearrange("c (s p q) -> c s p q", p=32, q=32)
                u_in = upat3[:, s0 : s0 + ST, :].unsqueeze(3).broadcast_to(
                    [128, ST, 32, 32]
                )
                v_in = vpat3[:, s0 : s0 + ST, :].unsqueeze(2).broadcast_to(
                    [128, ST, 32, 32]
                )
                nc.vector.tensor_tensor(pat4, u_in, v_in, mybir.AluOpType.mult)

                # conv1: K=128 block-diag matmul per 512 cols (all 16 taps)
                for half in range(ST * 2):
                    ps = ps1p.tile([128, 512], F32, tag="ps1", name="ps1t")
                    nc.tensor.matmul(
                        ps[:],
                        w1bd[:],
                        patches[:, 512 * half : 512 * (half + 1)],
                        start=True,
                        stop=True,
                    )
                    nc.scalar.activation(
                        x2[:, 512 * half : 512 * (half + 1)],
                        ps[:],
                        AF.Relu,
                        bias=biases[:, 0:1],
                    )

                if MAXL >= 2:
                    conv_layer(2, x2, x3, ps1p, "ps1")
                if MAXL >= 3:
                    conv_layer(3, x3, x4, ps2p, "ps2")
                if MAXL >= 4:
                    conv_layer(4, x4, x5, ps3p, "ps3", glob_s0=s0)

            # ---------------- conv5 + conv6 (global) ----------------
            if MAXL >= 5:
                conv_layer(5, x5, x6, ps2p, "ps2", st=NG)
            if MAXL >= 6:
                conv_layer(6, x6, y6, ps2p, "ps2", st=NG)

            # ---------------- head ----------------
            psh = ps3p.tile([128, 256], F32, tag="ps3", name="psh")
            nc.tensor.matmul(
                psh[0:4, 0:NG], wpbd[:], y6[:], start=True, stop=True
            )
            nc.scalar.activation(
                outsb[:],
                psh[0:4, 0:NG],
                AF.Sigmoid,
                bias=biases[0:4, 6:7],
            )
            nc.sync.dma_start(out_t[:], outsb[:])

    nc.compile()
    return nc


def _prep_weights(inputs):
    """Weight-derived device constants (identical on every core)."""
    import ml_dtypes

    w1 = np.asarray(inputs["conv1_w"], dtype=np.float32)
    b1 = np.asarray(inputs["conv1_b"], dtype=np.float32)
    wr = np.asarray(inputs["rest_w"], dtype=np.float32)
    br = np.asarray(inputs["rest_b"], dtype=np.float32)
    wp = np.asarray(inputs["pred_w"], dtype=np.float32)
    bp = np.asarray(inputs["pred_b"], dtype=np.float32)

    # R[src, (4*tbl + t)*128 + dst] with dst = 32g + 8a + 2b + d; the u-table
    # blocks (tbl=0) read deduplicated src row g%2, the v blocks src row g
    rmat = np.zeros((32, 8 * 128), dtype=ml_dtypes.float8_e4m3)
    for g in range(4):
        for a in range(4):
            for b in range(4):
                for dd in range(2):
                    dst = 32 * g + 8 * a + 2 * b + dd
                    rmat[g % 2, 128 * a + dst] = 1.0
                    rmat[g, 128 * (4 + b) + dst] = 1.0
    # conv1 block-diag: w1bd[32g + r, 32g' + co] = delta_gg' * w1[co,0,a,b]/2
    w1blk = np.zeros((32, 32), dtype=np.float16)  # [r=(8a+2b+d), cout]
    for a in range(4):
        for b in range(4):
            for dd in range(2):
                w1blk[8 * a + 2 * b + dd, :] = 0.5 * w1[:, 0, a, b]
    w1bd = np.zeros((128, 128), dtype=np.float16)
    for g in range(4):
        w1bd[32 * g : 32 * g + 32, 32 * g : 32 * g + 32] = w1blk
    # conv2..6 block-diag per tap
    wbd = np.zeros((128, 5 * 16 * 128), dtype=np.float16)
    for L in range(5):
        for a in range(4):
            for b in range(4):
                col0 = (L * 16 + 4 * a + b) * 128
                blkT = wr[L, :, :, a, b].T.astype(np.float16)  # [cin, cout]
                for g in range(4):
                    wbd[
                        32 * g : 32 * g + 32, col0 + 32 * g : col0 + 32 * g + 32
                    ] = blkT
    # head block-diag: wpbd[32g + c, g] = wp[0, c]
    wpbd = np.zeros((128, 4), dtype=np.float16)
    biases = np.zeros((128, 8), dtype=np.float32)
    for g in range(4):
        wpbd[32 * g : 32 * g + 32, g] = wp[0, :]
        biases[32 * g : 32 * g + 32, 0] = b1
        for L in range(5):
            biases[32 * g : 32 * g + 32, 1 + L] = br[L]
    biases[:, 6] = bp[0]
    return dict(rmat=rmat, w1bd=w1bd, wbd=wbd, wpbd=wpbd, biases=biases)


def _gather_u(inputs):
    """Global [16, NG*64] fp8 ug: rows 2c+h for core c, half h.

    Per core the device expands row h to partition groups h and h+2, so
    only the 512 distinct user rows upload.  Gathering from the fp32 table
    first and narrowing only the gathered rows avoids converting the 256MB
    user table every call.
    """
    import ml_dtypes

    idx_u = np.asarray(inputs["user"]).reshape(-1)
    uw = np.asarray(inputs["user_emb_w"])
    return (
        uw[idx_u]
        .astype(ml_dtypes.float8_e4m3)
        .reshape(2 * N_CORES, NG * 64)
    )


def _gather_v(inputs):
    """Global [32, NG*64] fp8 vg: item_pos rows for groups 0,1 of each
    core, item_neg rows for groups 2,3."""
    import ml_dtypes

    ipos = np.asarray(inputs["item_pos"]).reshape(N_CORES, 2, NG)
    ineg = np.asarray(inputs["item_neg"]).reshape(N_CORES, 2, NG)
    iw = np.asarray(inputs["item_emb_w"])
    idx_v = np.concatenate([ipos, ineg], axis=1).reshape(-1)
    return (
        iw[idx_v]
        .astype(ml_dtypes.float8_e4m3)
        .reshape(4 * N_CORES, NG * 64)
    )


_CACHED = {}
_WEIGHT_NAMES = ("rmat", "w1bd", "wbd", "wpbd", "biases")


def _get_runtime():
    """Build the Bass program + cached jitted executable once per process."""
    if "rt" in _CACHED:
        return _CACHED["rt"]

    import jax
    from jax.sharding import Mesh, NamedSharding, PartitionSpec

    import warnings

    with warnings.catch_warnings():
        warnings.simplefilter("ignore")
        try:
            from jax.experimental.shard_map import shard_map

            sm_kwargs = {"check_rep": False}
        except ImportError:
            from jax import shard_map

            sm_kwargs = {"check_vma": False}
    from concourse import mybir
    from concourse.bass2jax import (
        _bass_exec_p,
        install_neuronx_cc_hook,
        partition_id_tensor,
    )

    install_neuronx_cc_hook()
    nc = _build_program()

    partition_name = (
        nc.partition_id_tensor.name if nc.partition_id_tensor else None
    )
    in_names, out_names, out_avals, zero_shapes = [], [], [], []
    for alloc in nc.m.functions[0].allocations:
        if not isinstance(alloc, mybir.MemoryLocationSet):
            continue
        name = alloc.memorylocations[0].name
        if alloc.kind == "ExternalInput":
            if name != partition_name:
                in_names.append(name)
        elif alloc.kind == "ExternalOutput":
            out_names.append(name)
            shape = tuple(alloc.tensor_shape)
            dtype = mybir.dt.np(alloc.dtype)
            out_avals.append(jax.core.ShapedArray(shape, dtype))
            zero_shapes.append(((N_CORES * shape[0], *shape[1:]), dtype))
    n_params = len(in_names)
    n_outs = len(out_avals)
    all_names = in_names + out_names + (
        [partition_name] if partition_name else []
    )

    def _body(*args):
        operands = list(args)
        if partition_name is not None:
            operands.append(partition_id_tensor())
        outs = _bass_exec_p.bind(
            *operands,
            out_avals=tuple(out_avals),
            in_names=tuple(all_names),
            out_names=tuple(out_names),
            lowering_input_output_aliases=(),
            sim_require_finite=True,
            sim_require_nnan=True,
            nc=nc,
        )
        return tuple(outs)

    devices = jax.devices()[:N_CORES]
    assert len(devices) == N_CORES
    mesh = Mesh(np.asarray(devices), ("core",))
    sh = NamedSharding(mesh, PartitionSpec("core"))
    in_specs = (PartitionSpec("core"),) * (n_params + n_outs)
    out_specs = (PartitionSpec("core"),) * len(out_names)
    donate = tuple(range(n_params, n_params + n_outs))
    sharded = jax.jit(
        shard_map(
            _body,
            mesh=mesh,
            in_specs=in_specs,
            out_specs=out_specs,
            **sm_kwargs,
        ),
        donate_argnums=donate,
        keep_unused=True,
    )

    rt = dict(
        jax=jax,
        nc=nc,
        sharded=sharded,
        in_names=in_names,
        out_names=out_names,
        zero_shapes=zero_shapes,
        sh=sh,
        pool=ThreadPoolExecutor(max_workers=2 * N_CORES),
        weights_np=None,
        weights_dev=None,
        next_zeros=None,
    )
    _CACHED["rt"] = rt
    return rt


def _make_zeros(rt):
    jax, sh = rt["jax"], rt["sh"]
    return [
        jax.device_put(np.zeros(shape, dtype), sh)
        for shape, dtype in rt["zero_shapes"]
    ]


def _weight_hash(inputs):
    import hashlib

    h = hashlib.blake2b()
    for k in ("conv1_w", "conv1_b", "rest_w", "rest_b", "pred_w", "pred_b"):
        a = np.asarray(inputs[k])
        h.update(str(a.shape).encode())
        h.update(np.ascontiguousarray(a).tobytes())
    return h.digest()


def _run_device(inputs):
    rt = _get_runtime()
    jax, sh = rt["jax"], rt["sh"]

    # start streaming the embedding rows before anything else; u first so
    # its transfer overlaps the v gather
    ug_dev = jax.device_put(_gather_u(inputs), sh)
    vg_dev = jax.device_put(_gather_v(inputs), sh)

    # weight-derived constants stay device-resident; re-derive and re-upload
    # only when the caller passes different weight bytes
    wh = _weight_hash(inputs)
    if rt["weights_np"] != wh:
        rt["weights_np"] = None  # invalidate until the upload fully succeeds
        wnp = _prep_weights(inputs)
        rt["weights_dev"] = {
            k: jax.device_put(np.concatenate([wnp[k]] * N_CORES, axis=0), sh)
            for k in _WEIGHT_NAMES
        }
        rt["weights_np"] = wh

    per_call = {"ug": ug_dev, "vg": vg_dev}
    args = [
        per_call[name] if name in per_call else rt["weights_dev"][name]
        for name in rt["in_names"]
    ]
    zeros = rt["next_zeros"]
    rt["next_zeros"] = None  # never reuse a possibly-donated buffer on error
    if zeros is None:
        zeros = _make_zeros(rt)
    outs = rt["sharded"](*args, *zeros)

    # parallel per-shard fetch: one blocking point, ~RTT total
    out_global = outs[0]  # [N_CORES*4, NG] f32
    shards = out_global.addressable_shards
    datas = list(rt["pool"].map(lambda s: np.asarray(s.data), shards))
    res = np.empty((N_CORES * 4, NG), dtype=np.float32)
    for s, d in zip(shards, datas):
        r0 = s.index[0].start or 0
        res[r0 : r0 + d.shape[0]] = d
    # donated zero buffers were consumed; stage the next call's copy now
    # (after the fetch, so its upload never contends with the response)
    rt["next_zeros"] = _make_zeros(rt)
    return res


def kernel_with_stats(**inputs):
    try:
        per_core_out = _run_device(inputs).reshape(N_CORES, 4, NG)
    except Exception:
        # fall back to the stock (slow but battle-tested) runner
        from concourse.bass_utils import run_bass_kernel_spmd

        if "nc_slow" not in _CACHED:
            _CACHED["nc_slow"] = _build_program()
        wnp = _prep_weights(inputs)
        ug = _gather_u(inputs)
        vg = _gather_v(inputs)
        in_maps = [
            dict(
                ug=ug[2 * c : 2 * c + 2],
                vg=vg[4 * c : 4 * c + 4],
                **wnp,
            )
            for c in range(N_CORES)
        ]
        res = run_bass_kernel_spmd(
            _CACHED["nc_slow"], in_maps, core_ids=list(range(N_CORES))
        )
        per_core_out = np.stack([res.results[c]["out"] for c in range(N_CORES)])

    out1 = np.zeros((B, 1), dtype=np.float32)
    out2 = np.zeros((B, 1), dtype=np.float32)
    for c in range(N_CORES):
        o = per_core_out[c]  # [4, NG]
        out1[NB * c : NB * c + NB, 0] = o[0:2].reshape(-1)
        out2[NB * c : NB * c + NB, 0] = o[2:4].reshape(-1)
    return (out1, out2), None


def kernel(**inputs):
    out, _ = kernel_with_stats(**inputs)
    return out


# revision 19
# speedup vs baseline: 1.1010x; 1.0638x over previous
"""ConvNCF Trainium2 kernel (8 NeuronCores, data-parallel over batch).

Sharding: batch 4096 -> 8 cores x 512 samples.  Per core the device batch is
1024 rows ([512 pos | 512 neg]); rows are split into 4 partition groups
g = n // 256 of 32 channels each.  Each conv layer is a single K=128
block-diagonal matmul per (tap, column-chunk): lhsT is a [128,128] fp16
4x(32x32) block-diagonal weight, so all 4 groups' convolutions run in one PE
instruction (full-array MACs, 4x fewer instructions than per-group tiling).

The host performs only the embedding row lookup (the device runtime's
indirect-DMA gather scrambles multi-row-per-partition transfers, verified
empirically) and ships 96KB of gathered fp8-e4m3 rows per core (32KB user —
deduplicated, since groups 0,2 and 1,3 share rows — plus 64KB item; fp8
costs 1e-3 end-to-end rel err vs 3e-5 for fp16, well inside tolerance);
everything else runs on device:

1. R-permute matmuls expand the 4 gathered row-groups into the conv1 im2col
   u/v factor layout upat/vpat[32g + 8a + 2b + d, (s, p)] = u[n, 2p+a-1],
   using per-matmul shifted stride-2 windows for the tap offset.
2. A broadcast tensor_tensor builds conv1 outer-product patches
   patches[pi, (s,p,q)] = upat[pi,(s,p)] * vpat[pi,(s,q)], so one K=128
   block-diag matmul per 512 columns evaluates all 16 conv1 taps (host halves
   w1 to cancel the duplicated tap rows).
3. conv2..6 read UNPADDED fp16 activation tiles with stride-2 window APs;
   out-of-range edge taps simply skip those output columns (their zero-pad
   contribution is implicit in PSUM accumulation, started by the always-valid
   (1,1) tap).  ScalarE fuses bias+relu on PSUM->SBUF evacuation.
4. Head: one block-diag matmul + fused sigmoid, fp32 out [4, 256].

Runtime: every axon RPC costs ~70ms RTT, so the steady-state path keeps the
jitted executable and all weight-derived device inputs resident across calls
and blocks exactly once per call (parallel per-shard output fetch).  Only the
gathered embedding rows (2MB fp16) and the 32KB donated output-zero buffer
move per call; the zero buffer for the next call is uploaded during this
call's execution.  Weight-derived constants are fingerprinted each call and
re-uploaded if the caller passes different weights.
"""

from concurrent.futures import ThreadPoolExecutor

import numpy as np

B, D, NFM = 4096, 64, 32
N_CORES = 8
NB = B // N_CORES          # 512 samples per core
NDEV = 2 * NB              # 1024 device rows (pos branch then neg branch)
NG = NDEV // 4             # 256 rows per partition group
N_TILES = 32
ST = NG // N_TILES         # 8 slots per group per tile

IN_SIDE = {2: 32, 3: 16, 4: 8, 5: 4, 6: 2}   # unpadded input side per layer
OUT_SIDE = {1: 32, 2: 16, 3: 8, 4: 4, 5: 2, 6: 1}


def win1d(shift, isize, osize):
    """Valid out range [lo, hi) for in index 2*o + shift in [0, isize)."""
    lo = 0
    while 2 * lo + shift < 0:
        lo += 1
    hi = osize
    while hi > lo and 2 * (hi - 1) + shift >= isize:
        hi -= 1
    return lo, hi


# conv1 u/v factor windows over the 64-wide embedding rows
WIN = [(lambda lo_hi: (lo_hi[0], lo_hi[1], 2 * lo_hi[0] + a - 1))(win1d(a - 1, 64, 32))
       for a in range(4)]


def _build_program():
    MAXL = 9  # all 6 conv layers + head (debug knob, always full network)
    import concourse.bacc as bacc
    import concourse.tile as tile
    from concourse import mybir

    F8 = mybir.dt.float8e4
    F16 = mybir.dt.float16
    F32 = mybir.dt.float32
    AF = mybir.ActivationFunctionType

    nc = bacc.Bacc("TRN2", target_bir_lowering=False, name="convncf")

    # embeddings ship as fp8 e4m3 (1e-3 end-to-end rel err, half the bytes);
    # ug is deduplicated: per-core groups 0,2 and 1,3 share user rows, so only
    # 2 row-groups upload and rmat's u-blocks fan row g%2 out to both groups
    ug_t = nc.dram_tensor("ug", [2, NG * 64], F8, kind="ExternalInput")
    vg_t = nc.dram_tensor("vg", [4, NG * 64], F8, kind="ExternalInput")
    rmat_t = nc.dram_tensor("rmat", [32, 8 * 128], F8, kind="ExternalInput")
    w1bd_t = nc.dram_tensor("w1bd", [128, 128], F16, kind="ExternalInput")
    wbd_t = nc.dram_tensor("wbd", [128, 5 * 16 * 128], F16, kind="ExternalInput")
    wpbd_t = nc.dram_tensor("wpbd", [128, 4], F16, kind="ExternalInput")
    bias_t = nc.dram_tensor("biases", [128, 8], F32, kind="ExternalInput")
    out_t = nc.dram_tensor("out", [4, NG], F32, kind="ExternalOutput")

    with tile.TileContext(nc) as tc:
        with (
            tc.tile_pool(name="const", bufs=1) as constp,
            tc.tile_pool(name="glob", bufs=1) as globp,
            tc.tile_pool(name="work", bufs=2) as workp,
            tc.tile_pool(name="ps1", bufs=2, space="PSUM") as ps1p,
            tc.tile_pool(name="ps2", bufs=2, space="PSUM") as ps2p,
            tc.tile_pool(name="ps3", bufs=2, space="PSUM") as ps3p,
        ):
            w1bd = constp.tile([128, 128], F16, name="w1bd")
            wbd = constp.tile([128, 5 * 16 * 128], F16, name="wbd")
            wpbd = constp.tile([128, 4], F16, name="wpbd")
            biases = constp.tile([128, 8], F32, name="biases")
            upat = globp.tile([128, NG * 32], F16, name="upat")
            vpat = globp.tile([128, NG * 32], F16, name="vpat")
            x5 = globp.tile([128, NG * 16], F16, name="x5")   # conv5 in, 4x4
            x6 = globp.tile([128, NG * 4], F16, name="x6")    # conv6 in, 2x2
            y6 = globp.tile([128, NG], F16, name="y6")
            outsb = globp.tile([4, NG], F32, name="outsb")

            nc.gpsimd.memset(y6[:], 0.0)
            nc.sync.dma_start(w1bd[:], w1bd_t[:])
            nc.sync.dma_start(wbd[:], wbd_t[:])
            nc.sync.dma_start(wpbd[:], wpbd_t[:])
            nc.sync.dma_start(biases[:], bias_t[:])

            # ---- R-permute into upat/vpat (staging freed afterwards) ----
            with tc.tile_pool(name="pre", bufs=1) as prep:
                rmat = prep.tile([32, 8 * 128], F8, name="rmat")
                stg = prep.tile([128, NG * 64], F8, name="stg")
                nc.sync.dma_start(rmat[:], rmat_t[:])
                nc.gpsimd.memset(stg[:], 0.0)
                st3 = stg[:].rearrange("c (s e) -> c s e", e=64)
                SCH = 16  # slots per psum chunk -> 512 cols
                order = [1, 0, 2, 3]
                for tbl in range(2):
                    if tbl == 0:
                        nc.sync.dma_start(stg[0:2, :], ug_t[:])
                    else:
                        nc.sync.dma_start(stg[0:4, :], vg_t[:])
                    dstp = upat if tbl == 0 else vpat
                    for ch in range(NG // SCH):
                        s0 = ch * SCH
                        ps = ps2p.tile([128, 512], F32, tag="ps2", name="psr")
                        for i, t in enumerate(order):
                            lo, hi, o = WIN[t]
                            rhs = st3[
                                0:32, s0 : s0 + SCH, o : o + 2 * (hi - lo) - 1 : 2
                            ]
                            dst = ps[:].rearrange("c (s q) -> c s q", q=32)[
                                :, :, lo:hi
                            ]
                            nc.tensor.matmul(
                                dst,
                                rmat[
                                    :,
                                    128 * (4 * tbl + t) : 128 * (4 * tbl + t) + 128,
                                ],
                                rhs,
                                start=(i == 0),
                                stop=(i == 3),
                            )
                        nc.scalar.activation(
                            dstp[:, s0 * 32 : (s0 + SCH) * 32], ps[:], AF.Copy
                        )

            upat3 = upat[:].rearrange("c (s q) -> c s q", q=32)
            vpat3 = vpat[:].rearrange("c (s q) -> c s q", q=32)

            def w_l(layer, t):  # layer 2..6, tap t=4a+b -> [128,128] blockdiag
                c0 = ((layer - 2) * 16 + t) * 128
                return wbd[:, c0 : c0 + 128]

            # tap emission order: always-valid tap (a=1,b=1) first (start=True)
            TAP_ORDER = [5] + [t for t in range(16) if t != 5]

            def conv_layer(layer, xin, xout, psp, pstag, glob_s0=None, st=ST):
                """One block-diag K=128 matmul per (tap, chunk); windowed
                edge taps skip out-of-range columns."""
                isz = IN_SIDE[layer]
                osz = OUT_SIDE[layer]
                cols_slot = osz * osz
                total = st * cols_slot
                chw = min(total, 512)
                slots_ch = max(1, chw // cols_slot)
                nch = (total + chw - 1) // chw
                xi = xin[:].rearrange("c (s i) -> c s i", i=isz * isz)
                for ch in range(nch):
                    sa = ch * slots_ch
                    ps = psp.tile([128, chw], F32, tag=pstag, name="psc")
                    ps3 = ps[:].rearrange("c (s p q) -> c s p q", s=slots_ch, p=osz)
                    taps = []
                    for t in TAP_ORDER:
                        a, b = t // 4, t % 4
                        plo, phi = win1d(a - 1, isz, osz)
                        qlo, qhi = win1d(b - 1, isz, osz)
                        if plo < phi and qlo < qhi:
                            taps.append((t, a, b, plo, phi, qlo, qhi))
                    for i, (t, a, b, plo, phi, qlo, qhi) in enumerate(taps):
                        po = 2 * plo + a - 1
                        qo = 2 * qlo + b - 1
                        rhs = xi[:, sa : sa + slots_ch, :].rearrange(
                            "c s (p q) -> c s p q", p=isz
                        )[
                            :,
                            :,
                            po : po + 2 * (phi - plo) - 1 : 2,
                            qo : qo + 2 * (qhi - qlo) - 1 : 2,
                        ]
                        nc.tensor.matmul(
                            ps3[:, :, plo:phi, qlo:qhi],
                            w_l(layer, t),
                            rhs,
                            start=(i == 0),
                            stop=(i == len(taps) - 1),
                        )
                    base = (glob_s0 + sa) if glob_s0 is not None else sa
                    dst = xout[
                        :, base * (osz * osz) : (base + slots_ch) * (osz * osz)
                    ]
                    nc.scalar.activation(
                        dst,
                        ps[:],
                        AF.Relu,
                        bias=biases[:, layer - 1 : layer],
                    )

            # ---------------- tiled conv1..conv4 ----------------
            for ti in range(N_TILES):
                s0 = ti * ST
                patches = workp.tile(
                    [128, ST * 1024], F16, tag="patches", name="patches", bufs=1
                )
                x2 = workp.tile([128, ST * 1024], F16, tag="x2", name="x2")
                x3 = workp.tile([128, ST * 256], F16, tag="x3", name="x3", bufs=1)
                x4 = workp.tile([128, ST * 64], F16, tag="x4", name="x4", bufs=1)

                pat4 = patches[:].rearrange("c (s p q) -> c s p q", p=32, q=32)
                u_in = upat3[:, s0 : s0 + ST, :].unsqueeze(3).broadcast_to(
                    [128, ST, 32, 32]
                )
                v_in = vpat3[:, s0 : s0 + ST, :].unsqueeze(2).broadcast_to(
                    [128, ST, 32, 32]
                )
                nc.vector.tensor_tensor(pat4, u_in, v_in, mybir.AluOpType.mult)

                # conv1: K=128 block-diag matmul per 512 cols (all 16 taps)
                for half in range(ST * 2):
                    ps = ps1p.tile([128, 512], F32, tag="ps1", name="ps1t")
                    nc.tensor.matmul(
                        ps[:],
                        w1bd[:],
                        patches[:, 512 * half : 512 * (half + 1)],
                        start=True,
                        stop=True,
                    )
                    nc.scalar.activation(
                        x2[:, 512 * half : 512 * (half + 1)],
                        ps[:],
                        AF.Relu,
                        bias=biases[:, 0:1],
                    )

                if MAXL >= 2:
                    conv_layer(2, x2, x3, ps1p, "ps1")
                if MAXL >= 3:
                    conv_layer(3, x3, x4, ps2p, "ps2")
                if MAXL >= 4:
                    conv_layer(4, x4, x5, ps3p, "ps3", glob_s0=s0)

            # ---------------- conv5 + conv6 (global) ----------------
            if MAXL >= 5:
                conv_layer(5, x5, x6, ps2p, "ps2", st=NG)
            if MAXL >= 6:
                conv_layer(6, x6, y6, ps2p, "ps2", st=NG)

            # ---------------- head ----------------
            psh = ps3p.tile([128, 256], F32, tag="ps3", name="psh")
            nc.tensor.matmul(
                psh[0:4, 0:NG], wpbd[:], y6[:], start=True, stop=True
            )
            nc.scalar.activation(
                outsb[:],
                psh[0:4, 0:NG],
                AF.Sigmoid,
                bias=biases[0:4, 6:7],
            )
            nc.sync.dma_start(out_t[:], outsb[:])

    nc.compile()
    return nc


def _prep_weights(inputs):
    """Weight-derived device constants (identical on every core)."""
    import ml_dtypes

    w1 = np.asarray(inputs["conv1_w"], dtype=np.float32)
    b1 = np.asarray(inputs["conv1_b"], dtype=np.float32)
    wr = np.asarray(inputs["rest_w"], dtype=np.float32)
    br = np.asarray(inputs["rest_b"], dtype=np.float32)
    wp = np.asarray(inputs["pred_w"], dtype=np.float32)
    bp = np.asarray(inputs["pred_b"], dtype=np.float32)

    # R[src, (4*tbl + t)*128 + dst] with dst = 32g + 8a + 2b + d; the u-table
    # blocks (tbl=0) read deduplicated src row g%2, the v blocks src row g
    rmat = np.zeros((32, 8 * 128), dtype=ml_dtypes.float8_e4m3)
    for g in range(4):
        for a in range(4):
            for b in range(4):
                for dd in range(2):
                    dst = 32 * g + 8 * a + 2 * b + dd
                    rmat[g % 2, 128 * a + dst] = 1.0
                    rmat[g, 128 * (4 + b) + dst] = 1.0
    # conv1 block-diag: w1bd[32g + r, 32g' + co] = delta_gg' * w1[co,0,a,b]/2
    w1blk = np.zeros((32, 32), dtype=np.float16)  # [r=(8a+2b+d), cout]
    for a in range(4):
        for b in range(4):
            for dd in range(2):
                w1blk[8 * a + 2 * b + dd, :] = 0.5 * w1[:, 0, a, b]
    w1bd = np.zeros((128, 128), dtype=np.float16)
    for g in range(4):
        w1bd[32 * g : 32 * g + 32, 32 * g : 32 * g + 32] = w1blk
    # conv2..6 block-diag per tap
    wbd = np.zeros((128, 5 * 16 * 128), dtype=np.float16)
    for L in range(5):
        for a in range(4):
            for b in range(4):
                col0 = (L * 16 + 4 * a + b) * 128
                blkT = wr[L, :, :, a, b].T.astype(np.float16)  # [cin, cout]
                for g in range(4):
                    wbd[
                        32 * g : 32 * g + 32, col0 + 32 * g : col0 + 32 * g + 32
                    ] = blkT
    # head block-diag: wpbd[32g + c, g] = wp[0, c]
    wpbd = np.zeros((128, 4), dtype=np.float16)
    biases = np.zeros((128, 8), dtype=np.float32)
    for g in range(4):
        wpbd[32 * g : 32 * g + 32, g] = wp[0, :]
        biases[32 * g : 32 * g + 32, 0] = b1
        for L in range(5):
            biases[32 * g : 32 * g + 32, 1 + L] = br[L]
    biases[:, 6] = bp[0]
    return dict(rmat=rmat, w1bd=w1bd, wbd=wbd, wpbd=wpbd, biases=biases)


def _fp8_lut32():
    lut = _CACHED.get("fp8lut32")
    if lut is None:
        import ml_dtypes

        lut = (
            (np.arange(65536, dtype=np.uint64) << 16)
            .astype(np.uint32)
            .view(np.float32)
            .astype(ml_dtypes.float8_e4m3)
            .view(np.uint8)
        )
        _CACHED["fp8lut32"] = lut
    return lut


def _to_fp8(a):
    """fp32 -> fp8 e4m3 via a 64KB LUT on the top 16 bits.  4x faster than
    ml_dtypes astype; truncating the sticky bits misrounds <=1ulp on ~3% of
    values, invisible next to fp8's own quantization."""
    import ml_dtypes

    a32 = np.ascontiguousarray(a, dtype=np.float32)
    idx = a32.view(np.uint32) >> np.uint32(16)
    return _fp8_lut32().take(idx, mode="clip").view(ml_dtypes.float8_e4m3)


def _gather_u(inputs):
    """Global [16, NG*64] fp8 ug: rows 2c+h for core c, half h.

    Per core the device expands row h to partition groups h and h+2, so
    only the 512 distinct user rows upload.  Gathering from the fp32 table
    first and narrowing only the gathered rows avoids converting the 256MB
    user table every call.
    """
    idx_u = np.asarray(inputs["user"]).reshape(-1)
    uw = np.asarray(inputs["user_emb_w"])
    return _to_fp8(uw[idx_u]).reshape(2 * N_CORES, NG * 64)


def _gather_v(inputs):
    """Global [32, NG*64] fp8 vg: item_pos rows for groups 0,1 of each
    core, item_neg rows for groups 2,3."""
    ipos = np.asarray(inputs["item_pos"]).reshape(N_CORES, 2, NG)
    ineg = np.asarray(inputs["item_neg"]).reshape(N_CORES, 2, NG)
    iw = np.asarray(inputs["item_emb_w"])
    idx_v = np.concatenate([ipos, ineg], axis=1).reshape(-1)
    return _to_fp8(iw[idx_v]).reshape(4 * N_CORES, NG * 64)


_CACHED = {}
_WEIGHT_NAMES = ("rmat", "w1bd", "wbd", "wpbd", "biases")


def _get_runtime():
    """Build the Bass program + cached jitted executable once per process."""
    if "rt" in _CACHED:
        return _CACHED["rt"]

    import jax
    from jax.sharding import Mesh, NamedSharding, PartitionSpec

    import warnings

    with warnings.catch_warnings():
        warnings.simplefilter("ignore")
        try:
            from jax.experimental.shard_map import shard_map

            sm_kwargs = {"check_rep": False}
        except ImportError:
            from jax import shard_map

            sm_kwargs = {"check_vma": False}
    from concourse import mybir
    from concourse.bass2jax import (
        _bass_exec_p,
        install_neuronx_cc_hook,
        partition_id_tensor,
    )

    install_neuronx_cc_hook()
    nc = _build_program()

    partition_name = (
        nc.partition_id_tensor.name if nc.partition_id_tensor else None
    )
    in_names, out_names, out_avals, zero_shapes = [], [], [], []
    for alloc in nc.m.functions[0].allocations:
        if not isinstance(alloc, mybir.MemoryLocationSet):
            continue
        name = alloc.memorylocations[0].name
        if alloc.kind == "ExternalInput":
            if name != partition_name:
                in_names.append(name)
        elif alloc.kind == "ExternalOutput":
            out_names.append(name)
            shape = tuple(alloc.tensor_shape)
            dtype = mybir.dt.np(alloc.dtype)
            out_avals.append(jax.core.ShapedArray(shape, dtype))
            zero_shapes.append(((N_CORES * shape[0], *shape[1:]), dtype))
    n_params = len(in_names)
    n_outs = len(out_avals)
    all_names = in_names + out_names + (
        [partition_name] if partition_name else []
    )

    def _body(*args):
        operands = list(args)
        if partition_name is not None:
            operands.append(partition_id_tensor())
        outs = _bass_exec_p.bind(
            *operands,
            out_avals=tuple(out_avals),
            in_names=tuple(all_names),
            out_names=tuple(out_names),
            lowering_input_output_aliases=(),
            sim_require_finite=True,
            sim_require_nnan=True,
            nc=nc,
        )
        return tuple(outs)

    devices = jax.devices()[:N_CORES]
    assert len(devices) == N_CORES
    mesh = Mesh(np.asarray(devices), ("core",))
    sh = NamedSharding(mesh, PartitionSpec("core"))
    in_specs = (PartitionSpec("core"),) * (n_params + n_outs)
    out_specs = (PartitionSpec("core"),) * len(out_names)
    donate = tuple(range(n_params, n_params + n_outs))
    sharded = jax.jit(
        shard_map(
            _body,
            mesh=mesh,
            in_specs=in_specs,
            out_specs=out_specs,
            **sm_kwargs,
        ),
        donate_argnums=donate,
        keep_unused=True,
    )

    rt = dict(
        jax=jax,
        nc=nc,
        sharded=sharded,
        in_names=in_names,
        out_names=out_names,
        zero_shapes=zero_shapes,
        sh=sh,
        pool=ThreadPoolExecutor(max_workers=2 * N_CORES),
        weights_np=None,
        weights_dev=None,
        next_zeros=None,
    )
    _CACHED["rt"] = rt
    return rt


def _make_zeros(rt):
    jax, sh = rt["jax"], rt["sh"]
    return [
        jax.device_put(np.zeros(shape, dtype), sh)
        for shape, dtype in rt["zero_shapes"]
    ]


def _weight_hash(inputs):
    import hashlib

    h = hashlib.blake2b()
    for k in ("conv1_w", "conv1_b", "rest_w", "rest_b", "pred_w", "pred_b"):
        a = np.asarray(inputs[k])
        h.update(str(a.shape).encode())
        h.update(np.ascontiguousarray(a).tobytes())
    return h.digest()


def _run_device(inputs):
    rt = _get_runtime()
    jax, sh = rt["jax"], rt["sh"]

    # start streaming the embedding rows before anything else; u first so
    # its transfer overlaps the v gather
    ug_dev = jax.device_put(_gather_u(inputs), sh)
    vg_dev = jax.device_put(_gather_v(inputs), sh)

    # weight-derived constants stay device-resident; re-derive and re-upload
    # only when the caller passes different weight bytes
    wh = _weight_hash(inputs)
    if rt["weights_np"] != wh:
        rt["weights_np"] = None  # invalidate until the upload fully succeeds
        wnp = _prep_weights(inputs)
        rt["weights_dev"] = {
            k: jax.device_put(np.concatenate([wnp[k]] * N_CORES, axis=0), sh)
            for k in _WEIGHT_NAMES
        }
        rt["weights_np"] = wh

    per_call = {"ug": ug_dev, "vg": vg_dev}
    args = [
        per_call[name] if name in per_call else rt["weights_dev"][name]
        for name in rt["in_names"]
    ]
    zeros = rt["next_zeros"]
    rt["next_zeros"] = None  # never reuse a possibly-donated buffer on error
    if zeros is None:
        zeros = _make_zeros(rt)
    outs = rt["sharded"](*args, *zeros)

    # parallel per-shard fetch: one blocking point, ~RTT total
    out_global = outs[0]  # [N_CORES*4, NG] f32
    shards = out_global.addressable_shards
    datas = list(rt["pool"].map(lambda s: np.asarray(s.data), shards))
    res = np.empty((N_CORES * 4, NG), dtype=np.float32)
    for s, d in zip(shards, datas):
        r0 = s.index[0].start or 0
        res[r0 : r0 + d.shape[0]] = d
    # donated zero buffers were consumed; stage the next call's copy now
    # (after the fetch, so its upload never contends with the response)
    rt["next_zeros"] = _make_zeros(rt)
    return res


def kernel_with_stats(**inputs):
    try:
        per_core_out = _run_device(inputs).reshape(N_CORES, 4, NG)
    except Exception:
        # fall back to the stock (slow but battle-tested) runner
        from concourse.bass_utils import run_bass_kernel_spmd

        if "nc_slow" not in _CACHED:
            _CACHED["nc_slow"] = _build_program()
        wnp = _prep_weights(inputs)
        ug = _gather_u(inputs)
        vg = _gather_v(inputs)
        in_maps = [
            dict(
                ug=ug[2 * c : 2 * c + 2],
                vg=vg[4 * c : 4 * c + 4],
                **wnp,
            )
            for c in range(N_CORES)
        ]
        res = run_bass_kernel_spmd(
            _CACHED["nc_slow"], in_maps, core_ids=list(range(N_CORES))
        )
        per_core_out = np.stack([res.results[c]["out"] for c in range(N_CORES)])

    out1 = np.zeros((B, 1), dtype=np.float32)
    out2 = np.zeros((B, 1), dtype=np.float32)
    for c in range(N_CORES):
        o = per_core_out[c]  # [4, NG]
        out1[NB * c : NB * c + NB, 0] = o[0:2].reshape(-1)
        out2[NB * c : NB * c + NB, 0] = o[2:4].reshape(-1)
    return (out1, out2), None


def kernel(**inputs):
    out, _ = kernel_with_stats(**inputs)
    return out
